# revision 18
# baseline (speedup 1.0000x reference)
"""Trainium2 Bass kernel for nn_DialogueRNNCell (data-parallel over batch, 8 cores).

Sharding: batch axis (2048) split into 8 slices of 256 rows; GRU weights and the
attention vector are replicated.  Each core runs an identical Bass/Tile program.

Per-core layout: batch rows on SBUF partitions, two half-tiles of 128 rows.
The 64 MB/core global_hist stream is read once: a fused DVE scalar_tensor_tensor
computes the attention scores, and the softmax-weighted pooling accumulates in
PSUM via diag(exp(s_t)) @ hist_t matmuls (float32r, full PE rate).
"""

import sys

for _p in ("/opt/trn_rl_repo",):
    if _p not in sys.path:
        sys.path.insert(0, _p)

import numpy as np

import concourse.bass as bass
import concourse.tile as tile
from concourse import mybir
from concourse.vector_clock import ScopedClock

# ---- problem constants (hardcoded per spec) ----
B, P, T = 2048, 2, 128
U = G = DP = E = 512
H3 = 3 * 512
NCORES = 8
BL = B // NCORES  # 256 batch rows per core
TC = 2            # time steps per stream chunk
NCH = T // TC
F32 = mybir.dt.float32
F32R = mybir.dt.float32r
AX = mybir.AluOpType
AF = mybir.ActivationFunctionType

TRACE = False  # test harness can flip this to capture an NTFF profile

# free-dim offsets inside the packed `misc` tile [128, 1024]
O_ID = 0      # identity [128, 128]
O_WBC = 128   # attn_w broadcast [128, 512]
O_E = 640     # exp(scores) per half: [640:768] h=0, [768:896] h=1
O_PM = 896    # party mask cols: 896+h*2+p
O_DEN = 900   # 900+h
O_REC = 902   # 902+h
O_S = 904     # score scratch: 904 + h*4 + (c%2)*2 + i   (TC=2, chunk pairs)


# ---------------------------------------------------------------------------
# Workaround: walrus in this toolchain accepts only ONE sync wait on the
# CTRL-class drain emitted at TileContext exit.  Split the waits across a
# chain of drains.
# ---------------------------------------------------------------------------
def _patched_drain_and_barrier(self, tick_clock, wait_clock):
    nc = self.nc
    drain_inst = nc.sync.drain()
    wait_clock.add_sem_waits(
        drain_inst.ins, ScopedClock({None: tick_clock.global_clock})
    )
    inst = drain_inst.ins
    si = inst.sync_info
    maxw = 1
    if si is not None and len(si.on_wait) > maxw:
        waits = list(si.on_wait)
        SI = type(si)
        inst.sync_info = SI(on_wait=waits[:maxw], on_update=list(si.on_update))
        for k in range(maxw, len(waits), maxw):
            d2 = nc.sync.drain()
            d2.ins.sync_info = SI(on_wait=waits[k:k + maxw], on_update=[])
    nc.all_engine_barrier()
    assert self.sems is not None
    popped = nc._tile_sem_poison_stack.pop()
    assert popped is self._sem_poison
    nc.clear_and_free_semaphores(list(self.sems.allocated().values()))
    nc.all_engine_barrier()


tile.TileContext._drain_and_barrier = _patched_drain_and_barrier

_MAX_WAITS = 1
SPLIT_WAITS = True  # dev CoreSim runs need this off (NoOps lack fake sem updates)


def _split_excess_waits(nc, maxw=_MAX_WAITS):
    """This toolchain's walrus rejects instructions with more than `maxw`
    sync waits.  Move the excess onto NoOp instructions inserted directly
    before the offender on the same engine (waits are monotonic within a
    kernel, so waiting earlier in program order is safe)."""
    nseq = [0]
    for f in nc.m.functions:
        for blk in f.blocks:
            insts = list(blk.instructions)
            out = []
            changed = False
            for inst in insts:
                si = getattr(inst, "sync_info", None)
                if si is not None and len(si.on_wait) > maxw:
                    changed = True
                    waits = list(si.on_wait)
                    SI = type(si)
                    keep = len(waits) - maxw
                    for k in range(0, keep, maxw):
                        nop = mybir.InstNoOp(
                            name=f"waitnop_{nseq[0]}", ins=[], outs=[],
                            engine=inst.engine,
                        )
                        nseq[0] += 1
                        nop.sync_info = SI(on_wait=waits[k:k + maxw], on_update=[])
                        out.append(nop)
                    inst.sync_info = SI(on_wait=waits[keep:],
                                        on_update=list(si.on_update))
                out.append(inst)
            if changed:
                blk.instructions = out


# ---------------------------------------------------------------------------
# Device program
# ---------------------------------------------------------------------------
def _emit(nc, tc, ctx, io):
    from concourse.masks import make_identity

    hist, uT, pm, lps, les = io["hist"], io["uT"], io["pm"], io["lps"], io["les"]
    wgih, wghh, wpih, wphh, wlih, wlhh, weih, wehh, wbc = (
        io["wgih"], io["wghh"], io["wpih"], io["wphh"],
        io["wlih"], io["wlhh"], io["weih"], io["wehh"], io["wbc"],
    )
    og, op, oe, oa = io["og"], io["op"], io["oe"], io["oa"]

    pers = ctx.enter_context(tc.tile_pool(name="pers", bufs=1))
    histp = ctx.enter_context(tc.tile_pool(name="histp", bufs=6))
    scr = ctx.enter_context(tc.tile_pool(name="scr", bufs=1))
    dgp = ctx.enter_context(tc.tile_pool(name="dgp", bufs=2))
    wp = ctx.enter_context(tc.tile_pool(name="wp", bufs=6))
    xtp = ctx.enter_context(tc.tile_pool(name="xtp", bufs=5))
    gp = ctx.enter_context(tc.tile_pool(name="gp", bufs=10))
    outp = ctx.enter_context(tc.tile_pool(name="outp", bufs=7))
    nump = ctx.enter_context(tc.tile_pool(name="nump", bufs=2, space="PSUM"))
    gpsum = ctx.enter_context(tc.tile_pool(name="gpsum", bufs=4, space="PSUM"))
    trp = ctx.enter_context(tc.tile_pool(name="trp", bufs=1, space="PSUM"))

    # ---- packed constants / persistent state ----
    misc = pers.tile([128, 1024], F32, tag="misc", name="misc")
    ident = misc[:, O_ID:O_ID + 128]
    make_identity(nc, ident)
    nc.sync.dma_start(misc[:, O_WBC:O_WBC + G], wbc[:, :])
    for h in range(2):
        hs = h * 128
        for p in range(P):
            nc.sync.dma_start(
                misc[:, O_PM + h * 2 + p:O_PM + h * 2 + p + 1],
                pm[hs:hs + 128, p:p + 1],
            )

    def Ecol(h, t):
        return misc[:, O_E + h * 128 + t:O_E + h * 128 + t + 1]

    def Eslice(h):
        return misc[:, O_E + h * 128:O_E + (h + 1) * 128]

    def pmcol(h, p):
        return misc[:, O_PM + h * 2 + p:O_PM + h * 2 + p + 1]

    hprev = pers.tile([128, 1024], F32, tag="hprev", name="hprev")
    lesp = pers.tile([128, 1024], F32, tag="lesp", name="lesp")
    lpsp = [pers.tile([128, 1024], F32, tag=f"lps{p}", name=f"lps{p}")
            for p in range(P)]
    for h in range(2):
        hs = h * 128
        nc.sync.dma_start(hprev[:, h * 512:(h + 1) * 512],
                          hist[NCH - 1, h, :, TC - 1, :])
        nc.sync.dma_start(lesp[:, h * 512:(h + 1) * 512], les[hs:hs + 128, :])
        for p in range(P):
            nc.sync.dma_start(lpsp[p][:, h * 512:(h + 1) * 512], lps[hs:hs + 128, p, :])

    uTt = pers.tile([128, 1024], F32, tag="uT", name="uTt")
    nc.sync.dma_start(
        uTt.rearrange("p (c b) -> p c b", c=4).bitcast(F32R),
        uT.rearrange("(c p) b -> p c b", p=128).bitcast(F32R),
    )

    def uT_lhsT(c, h):
        return uTt[:, c * 256 + h * 128:c * 256 + h * 128 + 128]

    num = [nump.tile([128, G], F32, tag="num", name=f"num{h}") for h in range(2)]
    num_started = [False, False]

    # ---- helpers ----
    def transpose_pair(srcs, name):
        """srcs[h] = [128b, 512d] SBUF -> pair tile [128, 1024]:
        [:, h*512 + c*128 : ...] = chunk-c lhsT for half h."""
        dst = xtp.tile([128, 1024], F32, tag="xT", name=f"xT_{name}")
        for h in range(2):
            pt = trp.tile([128, 512], F32, tag="tr", name=f"tr_{name}{h}")
            for c in range(4):
                nc.tensor.transpose(
                    pt[:, c * 128:(c + 1) * 128],
                    srcs[h][:, c * 128:(c + 1) * 128], ident,
                )
            # rounds to f32r so the consuming matmuls pass BIR verification
            nc.scalar.copy(dst[:, h * 512:(h + 1) * 512].bitcast(F32R), pt)
        return dst

    def pair_lhsT(pair):
        return lambda c: lambda h: pair[:, h * 512 + c * 128:h * 512 + (c + 1) * 128]

    _wcache = {}

    def wtile(wdram, r0, n0):
        key = (id(wdram), r0, n0)
        if key in _wcache:
            return _wcache[key]
        t = wp.tile([128, 512], F32, tag="w", name=f"w_{len(_wcache)}")
        nc.sync.dma_start(t.bitcast(F32R),
                          wdram[r0:r0 + 128, n0:n0 + 512].bitcast(F32R))
        _wcache[key] = t
        return t

    def stream_pair(k):
        """Two chunks (4 time steps) per emission: scores on DVE, one batched
        exp on ACT, diag built on ACT, weighted accumulate on PE."""
        hts = {}
        for j in range(2):
            c = 2 * k + j
            for h in range(2):
                ht = histp.tile([128, TC, G], F32, tag="hist", name=f"ht{c}_{h}")
                nc.sync.dma_start(ht.bitcast(F32R), hist[c, h].bitcast(F32R))
                hts[j, h] = ht
                junk = scr.tile([128, 1024], F32, tag="junk", name=f"junk{c}_{h}")
                s0 = O_S + h * 4 + j * 2
                for i in range(TC):
                    nc.vector.scalar_tensor_tensor(
                        out=junk[:, i * 512:(i + 1) * 512], in0=ht[:, i, :],
                        scalar=1.0, in1=misc[:, O_WBC:O_WBC + G],
                        op0=AX.mult, op1=AX.mult,
                        accum_out=misc[:, s0 + i:s0 + i + 1],
                    )
        for h in range(2):
            nc.scalar.activation(
                misc[:, O_E + h * 128 + 4 * k:O_E + h * 128 + 4 * k + 4],
                misc[:, O_S + h * 4:O_S + h * 4 + 4], AF.Exp,
            )
        for h in range(2):
            dg = dgp.tile([128, 4, 128], F32, tag="diag", name=f"dg{k}_{h}")
            for j in range(2):
                for i in range(TC):
                    t = 4 * k + j * 2 + i
                    nc.scalar.activation(dg[:, j * 2 + i, :].bitcast(F32R), ident,
                                         AF.Copy, scale=Ecol(h, t))
                    nc.tensor.matmul(
                        num[h], lhsT=dg[:, j * 2 + i, :].bitcast(F32R),
                        rhs=hts[j, h][:, i, :].bitcast(F32R),
                        start=(not num_started[h]),
                        stop=(k == T // 4 - 1 and j == 1 and i == TC - 1),
                        skip_group_check=True,
                    )
                    num_started[h] = True

    def emit_gru(prefix, ih_srcs, hh_srcs_by_party, h_nat, out_pair_of_p):
        """One GRU cell, optionally party-batched.

        ih_srcs: [(wdram, row0, lhsT_fn: c-curried (h)->AP[128,128])] party-shared
        hh_srcs_by_party: {party: same}
        h_nat: {(h, party): AP [128,512]}
        out_pair_of_p: {party: pair tile}; hnew written to [:, h*512:...]
        """
        parties = sorted(hh_srcs_by_party)
        r_sb, z_sb = {}, {}
        for gname, n0 in (("r", 0), ("z", 512)):
            ps, nmm = {}, {}
            for h in range(2):
                for p in parties:
                    ps[h, p] = gpsum.tile([128, 512], F32, tag="gps",
                                          name=f"{prefix}_{gname}{h}{p}")
                    nmm[h, p] = 0
            total = {p: len(ih_srcs) + len(hh_srcs_by_party[p]) for p in parties}

            def mm(h, p, lh, wt):
                nc.tensor.matmul(
                    ps[h, p], lhsT=lh.bitcast(F32R), rhs=wt.bitcast(F32R),
                    start=(nmm[h, p] == 0), stop=(nmm[h, p] == total[p] - 1),
                    skip_group_check=True,
                )
                nmm[h, p] += 1

            for wdram, r0, lfn in ih_srcs:
                wt = wtile(wdram, r0, n0)
                for h in range(2):
                    for p in parties:
                        mm(h, p, lfn(h), wt)
            for p in parties:
                for wdram, r0, lfn in hh_srcs_by_party[p]:
                    wt = wtile(wdram, r0, n0)
                    for h in range(2):
                        mm(h, p, lfn(h), wt)
            for p in parties:
                g = gp.tile([128, 1024], F32, tag="gate",
                            name=f"{prefix}_{gname}sb{p}")
                for h in range(2):
                    nc.scalar.activation(g[:, h * 512:(h + 1) * 512], ps[h, p],
                                         AF.Sigmoid)
                (r_sb if gname == "r" else z_sb)[p] = g
        # i_n (party-shared)
        cps = {h: gpsum.tile([128, 512], F32, tag="gps", name=f"{prefix}_c{h}")
               for h in range(2)}
        for idx, (wdram, r0, lfn) in enumerate(ih_srcs):
            wt = wtile(wdram, r0, 1024)
            for h in range(2):
                nc.tensor.matmul(
                    cps[h], lhsT=lfn(h).bitcast(F32R), rhs=wt.bitcast(F32R),
                    start=(idx == 0), stop=(idx == len(ih_srcs) - 1),
                    skip_group_check=True,
                )
        # h_n per party + gate math
        for p in parties:
            dps = {h: gpsum.tile([128, 512], F32, tag="gps",
                                 name=f"{prefix}_d{h}{p}") for h in range(2)}
            srcs = hh_srcs_by_party[p]
            for idx, (wdram, r0, lfn) in enumerate(srcs):
                wt = wtile(wdram, r0, 1024)
                for h in range(2):
                    nc.tensor.matmul(
                        dps[h], lhsT=lfn(h).bitcast(F32R), rhs=wt.bitcast(F32R),
                        start=(idx == 0), stop=(idx == len(srcs) - 1),
                        skip_group_check=True,
                    )
            for h in range(2):
                hsl = slice(h * 512, (h + 1) * 512)
                t1 = gp.tile([128, 512], F32, tag="gate", name=f"{prefix}_t1{h}{p}")
                nc.vector.tensor_mul(t1, r_sb[p][:, hsl], dps[h])
                nin = gp.tile([128, 512], F32, tag="gate", name=f"{prefix}_ni{h}{p}")
                nc.vector.tensor_add(nin, t1, cps[h])
                n_t = gp.tile([128, 512], F32, tag="gate", name=f"{prefix}_n{h}{p}")
                nc.scalar.activation(n_t, nin, AF.Tanh)
                d2 = gp.tile([128, 512], F32, tag="gate", name=f"{prefix}_d2{h}{p}")
                nc.vector.tensor_sub(d2, h_nat[h, p], n_t)
                m = gp.tile([128, 512], F32, tag="gate", name=f"{prefix}_m{h}{p}")
                nc.vector.tensor_mul(m, z_sb[p][:, hsl], d2)
                nc.vector.tensor_add(out_pair_of_p[p][:, hsl], n_t, m)

    def onehot_blend(a_of_hp, name):
        """pair tile: [:, h*512:...] = sum_p pm[h,p] * a_of_hp(h,p)"""
        res = outp.tile([128, 1024], F32, tag="out", name=f"{name}_pair")
        for h in range(2):
            hsl = slice(h * 512, (h + 1) * 512)
            tmp = gp.tile([128, 512], F32, tag="gate", name=f"{name}_t{h}")
            nc.vector.tensor_scalar_mul(tmp, a_of_hp(h, 1), pmcol(h, 1))
            nc.vector.scalar_tensor_tensor(
                out=res[:, hsl], in0=a_of_hp(h, 0), scalar=pmcol(h, 0), in1=tmp,
                op0=AX.mult, op1=AX.add,
            )
        return res

    # ---- emission ----
    for k in range(2):
        stream_pair(k)

    # global GRU (runs while the stream continues)
    ss = onehot_blend(lambda h, p: lpsp[p][:, h * 512:(h + 1) * 512], "ss")
    ssT = transpose_pair([ss[:, 0:512], ss[:, 512:1024]], "ssT")
    hprevT = transpose_pair([hprev[:, 0:512], hprev[:, 512:1024]], "hpT")
    g_ih = [(wgih, c * 128, (lambda c_: lambda h: uT_lhsT(c_, h))(c)) for c in range(4)]
    g_ih += [(wgih, 512 + c * 128, pair_lhsT(ssT)(c)) for c in range(4)]
    g_hh = {0: [(wghh, c * 128, pair_lhsT(hprevT)(c)) for c in range(4)]}
    g_hnat = {(h, 0): hprev[:, h * 512:(h + 1) * 512] for h in range(2)}
    g_out = outp.tile([128, 1024], F32, tag="out", name="g_out")
    emit_gru("g", g_ih, g_hh, g_hnat, {0: g_out})
    for h in range(2):
        nc.sync.dma_start(og[h * 128:(h + 1) * 128, :], g_out[:, h * 512:(h + 1) * 512])

    # transpose lps / les while the stream continues (needed at the tail)
    lpsT = [transpose_pair([lpsp[p][:, 0:512], lpsp[p][:, 512:1024]], f"lpsT{p}")
            for p in range(P)]
    lesT = transpose_pair([lesp[:, 0:512], lesp[:, 512:1024]], "lesT")

    # rest of the stream
    for k in range(2, T // 4):
        stream_pair(k)

    # ---- attention epilogue: pooled + alpha ----
    alpha = outp.tile([128, 256], F32, tag="out", name="alpha")
    pooled = gp.tile([128, 1024], F32, tag="gate", name="pooled")
    for h in range(2):
        den = misc[:, O_DEN + h:O_DEN + h + 1]
        nc.vector.tensor_reduce(out=den, in_=Eslice(h), axis=mybir.AxisListType.X,
                                op=AX.add)
        rec = misc[:, O_REC + h:O_REC + h + 1]
        nc.vector.reciprocal(rec, den)
        nc.vector.tensor_scalar_mul(alpha[:, h * 128:(h + 1) * 128], Eslice(h), rec)
        nc.sync.dma_start(oa[h * 128:(h + 1) * 128, :], alpha[:, h * 128:(h + 1) * 128])
        nc.vector.tensor_scalar_mul(pooled[:, h * 512:(h + 1) * 512], num[h], rec)
    pooledT = transpose_pair([pooled[:, 0:512], pooled[:, 512:1024]], "plT")

    # ---- personal (speaker) GRU ----
    p_ih = [(wpih, c * 128, (lambda c_: lambda h: uT_lhsT(c_, h))(c)) for c in range(4)]
    p_ih += [(wpih, 512 + c * 128, pair_lhsT(pooledT)(c)) for c in range(4)]
    p_hh = {p: [(wphh, c * 128, pair_lhsT(lpsT[p])(c)) for c in range(4)]
            for p in range(P)}
    p_hnat = {(h, p): lpsp[p][:, h * 512:(h + 1) * 512]
              for h in range(2) for p in range(P)}
    cs = {p: outp.tile([128, 1024], F32, tag="out", name=f"cs{p}") for p in range(P)}
    emit_gru("p", p_ih, p_hh, p_hnat, cs)

    # ss2 = speaker row of cur_speaker (= emotion input; speaker rows of
    # cur_personal)
    ss2 = onehot_blend(lambda h, p: cs[p][:, h * 512:(h + 1) * 512], "ss2")
    ss2T = transpose_pair([ss2[:, 0:512], ss2[:, 512:1024]], "ss2T")

    # ---- emotion GRU (depends only on ss2) ----
    e_ih = [(weih, c * 128, pair_lhsT(ss2T)(c)) for c in range(4)]
    e_hh = {0: [(wehh, c * 128, pair_lhsT(lesT)(c)) for c in range(4)]}
    e_hnat = {(h, 0): lesp[:, h * 512:(h + 1) * 512] for h in range(2)}
    e_out = outp.tile([128, 1024], F32, tag="out", name="e_out")
    emit_gru("e", e_ih, e_hh, e_hnat, {0: e_out})
    for h in range(2):
        nc.sync.dma_start(oe[h * 128:(h + 1) * 128, :], e_out[:, h * 512:(h + 1) * 512])

    # ---- listener GRU ----
    l_ih = [(wlih, c * 128, (lambda c_: lambda h: uT_lhsT(c_, h))(c)) for c in range(4)]
    l_ih += [(wlih, 512 + c * 128, pair_lhsT(ss2T)(c)) for c in range(4)]
    l_hh = {p: [(wlhh, c * 128, pair_lhsT(lpsT[p])(c)) for c in range(4)]
            for p in range(P)}
    cl = {p: outp.tile([128, 1024], F32, tag="out", name=f"cl{p}") for p in range(P)}
    emit_gru("l", l_ih, l_hh, p_hnat, cl)

    # ---- scatter-blend cur_personal and store ----
    for p in range(P):
        for h in range(2):
            hsl = slice(h * 512, (h + 1) * 512)
            dd = gp.tile([128, 512], F32, tag="gate", name=f"bl_d{h}{p}")
            nc.vector.tensor_sub(dd, cs[p][:, hsl], cl[p][:, hsl])
            o = gp.tile([128, 512], F32, tag="gate", name=f"bl_o{h}{p}")
            nc.vector.scalar_tensor_tensor(
                out=o, in0=dd, scalar=pmcol(h, p), in1=cl[p][:, hsl],
                op0=AX.mult, op1=AX.add,
            )
            nc.sync.dma_start(op[h * 128:(h + 1) * 128, p, :], o)


def _build_program():
    nc = bass.Bass("TRN2", target_bir_lowering=False, debug=False)
    io = {}

    def din(name, shape):
        io[name] = nc.dram_tensor(name, list(shape), F32, kind="ExternalInput").ap()

    def dout(name, shape):
        io[name] = nc.dram_tensor(name, list(shape), F32, kind="ExternalOutput").ap()

    din("hist", (NCH, 2, 128, TC, G))
    din("uT", (U, BL))
    din("pm", (BL, P))
    din("lps", (BL, P, DP))
    din("les", (BL, E))
    din("wgih", (U + DP, H3))
    din("wghh", (G, H3))
    din("wpih", (U + G, H3))
    din("wphh", (DP, H3))
    din("wlih", (U + DP, H3))
    din("wlhh", (DP, H3))
    din("weih", (DP, H3))
    din("wehh", (E, H3))
    din("wbc", (128, G))
    dout("og", (BL, G))
    dout("op", (BL, P, DP))
    dout("oe", (BL, E))
    dout("oa", (BL, T))

    from contextlib import ExitStack
    with tile.TileContext(nc) as tc:
        with ExitStack() as ctx:
            _emit(nc, tc, ctx, io)
    if SPLIT_WAITS:
        _split_excess_waits(nc)
    return nc


_NC = None


def _get_program():
    global _NC
    if _NC is None:
        _NC = _build_program()
    return _NC


def kernel(utterance, party_mask, global_hist, last_personal_state,
           last_emotion_state,
           wg_ih, wg_hh, bg_ih, bg_hh, wp_ih, wp_hh, bp_ih, bp_hh,
           wl_ih, wl_hh, bl_ih, bl_hh, we_ih, we_hh, be_ih, be_hh, attn_w):
    asnp = lambda x: np.ascontiguousarray(np.asarray(x), dtype=np.float32)
    utterance = asnp(utterance)
    party_mask = asnp(party_mask)
    global_hist = asnp(global_hist)
    lps = asnp(last_personal_state)
    les = asnp(last_emotion_state)
    for b in (bg_ih, bg_hh, bp_ih, bp_hh, bl_ih, bl_hh, be_ih, be_hh):
        if np.abs(np.asarray(b)).max() != 0.0:
            raise NotImplementedError("nonzero GRU biases not supported")

    shared = {
        "wgih": asnp(np.asarray(wg_ih).T),
        "wghh": asnp(np.asarray(wg_hh).T),
        "wpih": asnp(np.asarray(wp_ih).T),
        "wphh": asnp(np.asarray(wp_hh).T),
        "wlih": asnp(np.asarray(wl_ih).T),
        "wlhh": asnp(np.asarray(wl_hh).T),
        "weih": asnp(np.asarray(we_ih).T),
        "wehh": asnp(np.asarray(we_hh).T),
        "wbc": asnp(np.tile(np.asarray(attn_w).reshape(1, G), (128, 1))),
    }

    in_maps = []
    for c in range(NCORES):
        sl = slice(c * BL, (c + 1) * BL)
        m = dict(shared)
        m["hist"] = asnp(
            global_hist[:, sl, :]
            .reshape(NCH, TC, 2, 128, G).transpose(0, 2, 3, 1, 4))
        m["uT"] = asnp(utterance[sl].T)
        m["pm"] = asnp(party_mask[sl])
        m["lps"] = asnp(lps[sl])
        m["les"] = asnp(les[sl])
        in_maps.append(m)

    nc = _get_program()
    from concourse.bass_utils import run_bass_kernel_spmd
    res = run_bass_kernel_spmd(
        nc, in_maps, core_ids=list(range(NCORES)), trace=TRACE
    )
    kernel.last_result = res

    og = np.concatenate([res.results[c]["og"] for c in range(NCORES)], axis=0)
    opd = np.concatenate([res.results[c]["op"] for c in range(NCORES)], axis=0)
    oe = np.concatenate([res.results[c]["oe"] for c in range(NCORES)], axis=0)
    oa = np.concatenate([res.results[c]["oa"] for c in range(NCORES)], axis=0)
    return og, opd, oe, oa[:, None, :]


# revision 20
# speedup vs baseline: 1.2924x; 1.2924x over previous
"""Trainium2 Bass kernel for nn_DialogueRNNCell (data-parallel over batch, 8 cores).

Sharding: batch axis (2048) split into 8 slices of 256 rows; GRU weights and the
attention vector are replicated.  Each core runs an identical Bass/Tile program.

Per-core layout: batch rows on SBUF partitions, two half-tiles of 128 rows.
The 64 MB/core global_hist stream is read once: a fused DVE scalar_tensor_tensor
computes the attention scores, and the softmax-weighted pooling accumulates in
PSUM via diag(exp(s_t)) @ hist_t matmuls (float32r, full PE rate).
"""

import sys

for _p in ("/opt/trn_rl_repo",):
    if _p not in sys.path:
        sys.path.insert(0, _p)

import numpy as np

import concourse.bass as bass
import concourse.tile as tile
from concourse import mybir
from concourse.vector_clock import ScopedClock

# ---- problem constants (hardcoded per spec) ----
B, P, T = 2048, 2, 128
U = G = DP = E = 512
H3 = 3 * 512
NCORES = 8
BL = B // NCORES  # 256 batch rows per core
TC = 2            # time steps per stream chunk
NCH = T // TC
F32 = mybir.dt.float32
F32R = mybir.dt.float32r
AX = mybir.AluOpType
AF = mybir.ActivationFunctionType

TRACE = False  # test harness can flip this to capture an NTFF profile

# free-dim offsets inside the packed `misc` tile [128, 1024]
O_ID = 0      # identity [128, 128]
O_WBC = 128   # attn_w broadcast [128, 512]
O_E = 640     # exp(scores) per half: [640:768] h=0, [768:896] h=1
O_PM = 896    # party mask cols: 896+h*2+p
O_DEN = 900   # 900+h
O_REC = 902   # 902+h
O_S = 904     # score scratch: 904 + h*4 + (c%2)*2 + i   (TC=2, chunk pairs)


# ---------------------------------------------------------------------------
# Workaround: walrus in this toolchain accepts only ONE sync wait on the
# CTRL-class drain emitted at TileContext exit.  Split the waits across a
# chain of drains.
# ---------------------------------------------------------------------------
def _patched_drain_and_barrier(self, tick_clock, wait_clock):
    nc = self.nc
    drain_inst = nc.sync.drain()
    wait_clock.add_sem_waits(
        drain_inst.ins, ScopedClock({None: tick_clock.global_clock})
    )
    inst = drain_inst.ins
    si = inst.sync_info
    maxw = 1
    if si is not None and len(si.on_wait) > maxw:
        waits = list(si.on_wait)
        SI = type(si)
        inst.sync_info = SI(on_wait=waits[:maxw], on_update=list(si.on_update))
        for k in range(maxw, len(waits), maxw):
            d2 = nc.sync.drain()
            d2.ins.sync_info = SI(on_wait=waits[k:k + maxw], on_update=[])
    nc.all_engine_barrier()
    assert self.sems is not None
    popped = nc._tile_sem_poison_stack.pop()
    assert popped is self._sem_poison
    nc.clear_and_free_semaphores(list(self.sems.allocated().values()))
    nc.all_engine_barrier()


tile.TileContext._drain_and_barrier = _patched_drain_and_barrier

_MAX_WAITS = 1
SPLIT_WAITS = True  # dev CoreSim runs need this off (NoOps lack fake sem updates)


def _split_excess_waits(nc, maxw=_MAX_WAITS):
    """This toolchain's walrus rejects instructions with more than `maxw`
    sync waits.  Move the excess onto NoOp instructions inserted directly
    before the offender on the same engine (waits are monotonic within a
    kernel, so waiting earlier in program order is safe)."""
    nseq = [0]
    for f in nc.m.functions:
        for blk in f.blocks:
            insts = list(blk.instructions)
            out = []
            changed = False
            for inst in insts:
                si = getattr(inst, "sync_info", None)
                if si is not None and len(si.on_wait) > maxw:
                    changed = True
                    waits = list(si.on_wait)
                    SI = type(si)
                    keep = len(waits) - maxw
                    for k in range(0, keep, maxw):
                        nop = mybir.InstNoOp(
                            name=f"waitnop_{nseq[0]}", ins=[], outs=[],
                            engine=inst.engine,
                        )
                        nseq[0] += 1
                        nop.sync_info = SI(on_wait=waits[k:k + maxw], on_update=[])
                        out.append(nop)
                    inst.sync_info = SI(on_wait=waits[keep:],
                                        on_update=list(si.on_update))
                out.append(inst)
            if changed:
                blk.instructions = out


# ---------------------------------------------------------------------------
# Device program
# ---------------------------------------------------------------------------
def _emit(nc, tc, ctx, io):
    from concourse.masks import make_identity

    hist, uT, pm, lps, les = io["hist"], io["uT"], io["pm"], io["lps"], io["les"]
    wgih, wghh, wpih, wphh, wlih, wlhh, weih, wehh, wbc = (
        io["wgih"], io["wghh"], io["wpih"], io["wphh"],
        io["wlih"], io["wlhh"], io["weih"], io["wehh"], io["wbc"],
    )
    og, op, oe, oa = io["og"], io["op"], io["oe"], io["oa"]

    pers = ctx.enter_context(tc.tile_pool(name="pers", bufs=1))
    histp = ctx.enter_context(tc.tile_pool(name="histp", bufs=8))
    scr = ctx.enter_context(tc.tile_pool(name="scr", bufs=1))
    dgp = ctx.enter_context(tc.tile_pool(name="dgp", bufs=2))
    wp = ctx.enter_context(tc.tile_pool(name="wp", bufs=10))
    xtp = ctx.enter_context(tc.tile_pool(name="xtp", bufs=5))
    gp = ctx.enter_context(tc.tile_pool(name="gp", bufs=9))
    outp = ctx.enter_context(tc.tile_pool(name="outp", bufs=6))
    nump = ctx.enter_context(tc.tile_pool(name="nump", bufs=2, space="PSUM"))
    gpsum = ctx.enter_context(tc.tile_pool(name="gpsum", bufs=4, space="PSUM"))
    trp = ctx.enter_context(tc.tile_pool(name="trp", bufs=1, space="PSUM"))

    # ---- packed constants / persistent state ----
    misc = pers.tile([128, 1024], F32, tag="misc", name="misc")
    ident = misc[:, O_ID:O_ID + 128]
    make_identity(nc, ident)
    nc.sync.dma_start(misc[:, O_WBC:O_WBC + G], wbc[:, :])
    for h in range(2):
        hs = h * 128
        for p in range(P):
            nc.sync.dma_start(
                misc[:, O_PM + h * 2 + p:O_PM + h * 2 + p + 1],
                pm[hs:hs + 128, p:p + 1],
            )

    etile = pers.tile([128, 256], F32, tag="etile", name="etile")
    sdr = pers.tile([128, 16], F32, tag="sdr", name="sdr")

    def Ecol(h, t):
        return etile[:, h * 128 + t:h * 128 + t + 1]

    def Eslice(h):
        return etile[:, h * 128:(h + 1) * 128]

    def pmcol(h, p):
        return misc[:, O_PM + h * 2 + p:O_PM + h * 2 + p + 1]

    hprev = pers.tile([128, 1024], F32, tag="hprev", name="hprev")
    lesp = pers.tile([128, 1024], F32, tag="lesp", name="lesp")
    lpsp = [pers.tile([128, 1024], F32, tag=f"lps{p}", name=f"lps{p}")
            for p in range(P)]
    for h in range(2):
        hs = h * 128
        nc.sync.dma_start(hprev[:, h * 512:(h + 1) * 512],
                          hist[NCH - 1, h, :, TC - 1, :])
        nc.sync.dma_start(lesp[:, h * 512:(h + 1) * 512], les[hs:hs + 128, :])
        for p in range(P):
            nc.sync.dma_start(lpsp[p][:, h * 512:(h + 1) * 512], lps[hs:hs + 128, p, :])

    uTt = pers.tile([128, 1024], F32, tag="uT", name="uTt")
    nc.sync.dma_start(
        uTt.rearrange("p (c b) -> p c b", c=4).bitcast(F32R),
        uT.rearrange("(c p) b -> p c b", p=128).bitcast(F32R),
    )

    def uT_lhsT(c, h):
        return uTt[:, c * 256 + h * 128:c * 256 + h * 128 + 128]

    num = [nump.tile([128, G], F32, tag="num", name=f"num{h}") for h in range(2)]
    num_started = [False, False]

    # ---- helpers ----
    def transpose_pair(srcs, name):
        """srcs[h] = [128b, 512d] SBUF -> pair tile [128, 1024]:
        [:, h*512 + c*128 : ...] = chunk-c lhsT for half h."""
        dst = xtp.tile([128, 1024], F32, tag="xT", name=f"xT_{name}")
        for h in range(2):
            pt = trp.tile([128, 512], F32, tag="tr", name=f"tr_{name}{h}")
            for c in range(4):
                nc.tensor.transpose(
                    pt[:, c * 128:(c + 1) * 128],
                    srcs[h][:, c * 128:(c + 1) * 128], ident,
                )
            # rounds to f32r so the consuming matmuls pass BIR verification
            nc.scalar.copy(dst[:, h * 512:(h + 1) * 512].bitcast(F32R), pt)
        return dst

    def pair_lhsT(pair):
        return lambda c: lambda h: pair[:, h * 512 + c * 128:h * 512 + (c + 1) * 128]

    _wcache = {}

    def wtile(wdram, r0, n0):
        key = (id(wdram), r0, n0)
        if key in _wcache:
            return _wcache[key]
        t = wp.tile([128, 512], F32, tag="w", name=f"w_{len(_wcache)}")
        nc.sync.dma_start(t.bitcast(F32R),
                          wdram[r0:r0 + 128, n0:n0 + 512].bitcast(F32R))
        _wcache[key] = t
        return t

    def stream_pair(k):
        """Two chunks (4 time steps) per emission: scores on DVE, one batched
        exp on ACT, diag built on ACT, weighted accumulate on PE."""
        hts = {}
        for j in range(2):
            c = 2 * k + j
            for h in range(2):
                ht = histp.tile([128, TC, G], F32, tag="hist", name=f"ht{c}_{h}")
                nc.sync.dma_start(ht.bitcast(F32R), hist[c, h].bitcast(F32R))
                hts[j, h] = ht
                junk = scr.tile([128, 1024], F32, tag="junk", name=f"junk{c}_{h}")
                s0 = 8 * (k % 2) + h * 4 + j * 2
                for i in range(TC):
                    nc.vector.scalar_tensor_tensor(
                        out=junk[:, i * 512:(i + 1) * 512], in0=ht[:, i, :],
                        scalar=1.0, in1=misc[:, O_WBC:O_WBC + G],
                        op0=AX.mult, op1=AX.mult,
                        accum_out=sdr[:, s0 + i:s0 + i + 1],
                    )
        for h in range(2):
            nc.scalar.activation(
                etile[:, h * 128 + 4 * k:h * 128 + 4 * k + 4],
                sdr[:, 8 * (k % 2) + h * 4:8 * (k % 2) + h * 4 + 4], AF.Exp,
            )
        for h in range(2):
            dg = dgp.tile([128, 4, 128], F32, tag="diag", name=f"dg{k}_{h}")
            for j in range(2):
                for i in range(TC):
                    t = 4 * k + j * 2 + i
                    nc.scalar.activation(dg[:, j * 2 + i, :].bitcast(F32R), ident,
                                         AF.Copy, scale=Ecol(h, t))
                    nc.tensor.matmul(
                        num[h], lhsT=dg[:, j * 2 + i, :].bitcast(F32R),
                        rhs=hts[j, h][:, i, :].bitcast(F32R),
                        start=(not num_started[h]),
                        stop=(k == T // 4 - 1 and j == 1 and i == TC - 1),
                        skip_group_check=True,
                    )
                    num_started[h] = True

    def gru_phases(prefix, ih_srcs, hh_srcs_by_party, h_nat, out_pair_of_p):
        """One GRU cell as a list of phase closures so the caller can
        interleave them with stream pairs (keeps the PE queue short)."""
        parties = sorted(hh_srcs_by_party)
        st = {"r": {}, "z": {}, "c": None}

        def rz_phase(gname, n0):
            ps, nmm = {}, {}
            for h in range(2):
                for p in parties:
                    ps[h, p] = gpsum.tile([128, 512], F32, tag="gps",
                                          name=f"{prefix}_{gname}{h}{p}")
                    nmm[h, p] = 0
            total = {p: len(ih_srcs) + len(hh_srcs_by_party[p]) for p in parties}

            def mm(h, p, lh, wt):
                nc.tensor.matmul(
                    ps[h, p], lhsT=lh.bitcast(F32R), rhs=wt.bitcast(F32R),
                    start=(nmm[h, p] == 0), stop=(nmm[h, p] == total[p] - 1),
                    skip_group_check=True,
                )
                nmm[h, p] += 1

            for wdram, r0, lfn in ih_srcs:
                wt = wtile(wdram, r0, n0)
                for h in range(2):
                    for p in parties:
                        mm(h, p, lfn(h), wt)
            for p in parties:
                for wdram, r0, lfn in hh_srcs_by_party[p]:
                    wt = wtile(wdram, r0, n0)
                    for h in range(2):
                        mm(h, p, lfn(h), wt)
            for p in parties:
                g = gp.tile([128, 1024], F32, tag="gate",
                            name=f"{prefix}_{gname}sb{p}")
                for h in range(2):
                    nc.scalar.activation(g[:, h * 512:(h + 1) * 512], ps[h, p],
                                         AF.Sigmoid)
                st[gname][p] = g

        def ni_phase():
            cps = {h: gpsum.tile([128, 512], F32, tag="gps",
                                 name=f"{prefix}_c{h}") for h in range(2)}
            for idx, (wdram, r0, lfn) in enumerate(ih_srcs):
                wt = wtile(wdram, r0, 1024)
                for h in range(2):
                    nc.tensor.matmul(
                        cps[h], lhsT=lfn(h).bitcast(F32R), rhs=wt.bitcast(F32R),
                        start=(idx == 0), stop=(idx == len(ih_srcs) - 1),
                        skip_group_check=True,
                    )
            st["c"] = cps

        def nh_phase():
            cps = st["c"]
            for p in parties:
                dps = {h: gpsum.tile([128, 512], F32, tag="gps",
                                     name=f"{prefix}_d{h}{p}") for h in range(2)}
                srcs = hh_srcs_by_party[p]
                for idx, (wdram, r0, lfn) in enumerate(srcs):
                    wt = wtile(wdram, r0, 1024)
                    for h in range(2):
                        nc.tensor.matmul(
                            dps[h], lhsT=lfn(h).bitcast(F32R), rhs=wt.bitcast(F32R),
                            start=(idx == 0), stop=(idx == len(srcs) - 1),
                            skip_group_check=True,
                        )
                for h in range(2):
                    hsl = slice(h * 512, (h + 1) * 512)
                    t1 = gp.tile([128, 512], F32, tag="gate",
                                 name=f"{prefix}_t1{h}{p}")
                    nc.vector.tensor_mul(t1, st["r"][p][:, hsl], dps[h])
                    nin = gp.tile([128, 512], F32, tag="gate",
                                  name=f"{prefix}_ni{h}{p}")
                    nc.vector.tensor_add(nin, t1, cps[h])
                    n_t = gp.tile([128, 512], F32, tag="gate",
                                  name=f"{prefix}_n{h}{p}")
                    nc.scalar.activation(n_t, nin, AF.Tanh)
                    d2 = gp.tile([128, 512], F32, tag="gate",
                                 name=f"{prefix}_d2{h}{p}")
                    nc.vector.tensor_sub(d2, h_nat[h, p], n_t)
                    m = gp.tile([128, 512], F32, tag="gate",
                                name=f"{prefix}_m{h}{p}")
                    nc.vector.tensor_mul(m, st["z"][p][:, hsl], d2)
                    nc.vector.tensor_add(out_pair_of_p[p][:, hsl], n_t, m)

        return [lambda: rz_phase("r", 0), lambda: rz_phase("z", 512),
                ni_phase, nh_phase]

    def emit_gru(prefix, ih_srcs, hh_srcs_by_party, h_nat, out_pair_of_p):
        for ph in gru_phases(prefix, ih_srcs, hh_srcs_by_party, h_nat,
                             out_pair_of_p):
            ph()

    def onehot_blend(a_of_hp, name):
        """pair tile: [:, h*512:...] = sum_p pm[h,p] * a_of_hp(h,p)"""
        res = outp.tile([128, 1024], F32, tag="out", name=f"{name}_pair")
        for h in range(2):
            hsl = slice(h * 512, (h + 1) * 512)
            tmp = gp.tile([128, 512], F32, tag="gate", name=f"{name}_t{h}")
            nc.vector.tensor_scalar_mul(tmp, a_of_hp(h, 1), pmcol(h, 1))
            nc.vector.scalar_tensor_tensor(
                out=res[:, hsl], in0=a_of_hp(h, 0), scalar=pmcol(h, 0), in1=tmp,
                op0=AX.mult, op1=AX.add,
            )
        return res

    # ---- emission: stream pairs with the global GRU + transposes woven in ---
    stream_pair(0)
    ss = onehot_blend(lambda h, p: lpsp[p][:, h * 512:(h + 1) * 512], "ss")
    ssT = transpose_pair([ss[:, 0:512], ss[:, 512:1024]], "ssT")
    hprevT = transpose_pair([hprev[:, 0:512], hprev[:, 512:1024]], "hpT")
    stream_pair(1)
    g_ih = [(wgih, c * 128, (lambda c_: lambda h: uT_lhsT(c_, h))(c)) for c in range(4)]
    g_ih += [(wgih, 512 + c * 128, pair_lhsT(ssT)(c)) for c in range(4)]
    g_hh = {0: [(wghh, c * 128, pair_lhsT(hprevT)(c)) for c in range(4)]}
    g_hnat = {(h, 0): hprev[:, h * 512:(h + 1) * 512] for h in range(2)}
    g_out = outp.tile([128, 1024], F32, tag="out", name="g_out")
    g_phases = gru_phases("g", g_ih, g_hh, g_hnat, {0: g_out})
    lpsT = [None, None]
    lesT = None
    for k in range(2, T // 4):
        stream_pair(k)
        if k - 2 < len(g_phases):
            g_phases[k - 2]()
            if k - 2 == len(g_phases) - 1:
                for h in range(2):
                    nc.sync.dma_start(og[h * 128:(h + 1) * 128, :],
                                      g_out[:, h * 512:(h + 1) * 512])
        elif k == 6:
            lpsT[0] = transpose_pair([lpsp[0][:, 0:512], lpsp[0][:, 512:1024]],
                                     "lpsT0")
        elif k == 7:
            lpsT[1] = transpose_pair([lpsp[1][:, 0:512], lpsp[1][:, 512:1024]],
                                     "lpsT1")
        elif k == 8:
            lesT = transpose_pair([lesp[:, 0:512], lesp[:, 512:1024]], "lesT")

    # ---- attention epilogue: pooled + alpha ----
    drr = pers.tile([128, 4], F32, tag="drr", name="drr")
    alpha = gp.tile([128, 256], F32, tag="gate", name="alpha")
    pooled = gp.tile([128, 1024], F32, tag="gate", name="pooled")
    for h in range(2):
        den = drr[:, h:h + 1]
        nc.vector.tensor_reduce(out=den, in_=Eslice(h), axis=mybir.AxisListType.X,
                                op=AX.add)
        rec = drr[:, 2 + h:3 + h]
        nc.vector.reciprocal(rec, den)
        nc.vector.tensor_scalar_mul(alpha[:, h * 128:(h + 1) * 128], Eslice(h), rec)
        nc.sync.dma_start(oa[h * 128:(h + 1) * 128, :], alpha[:, h * 128:(h + 1) * 128])
        nc.vector.tensor_scalar_mul(pooled[:, h * 512:(h + 1) * 512], num[h], rec)
    pooledT = transpose_pair([pooled[:, 0:512], pooled[:, 512:1024]], "plT")

    # ---- personal (speaker) GRU ----
    p_ih = [(wpih, c * 128, (lambda c_: lambda h: uT_lhsT(c_, h))(c)) for c in range(4)]
    p_ih += [(wpih, 512 + c * 128, pair_lhsT(pooledT)(c)) for c in range(4)]
    p_hh = {p: [(wphh, c * 128, pair_lhsT(lpsT[p])(c)) for c in range(4)]
            for p in range(P)}
    p_hnat = {(h, p): lpsp[p][:, h * 512:(h + 1) * 512]
              for h in range(2) for p in range(P)}
    cs = {p: outp.tile([128, 1024], F32, tag="out", name=f"cs{p}") for p in range(P)}
    emit_gru("p", p_ih, p_hh, p_hnat, cs)

    # ss2 = speaker row of cur_speaker (= emotion input; speaker rows of
    # cur_personal)
    ss2 = onehot_blend(lambda h, p: cs[p][:, h * 512:(h + 1) * 512], "ss2")
    ss2T = transpose_pair([ss2[:, 0:512], ss2[:, 512:1024]], "ss2T")

    # ---- emotion GRU (depends only on ss2) ----
    e_ih = [(weih, c * 128, pair_lhsT(ss2T)(c)) for c in range(4)]
    e_hh = {0: [(wehh, c * 128, pair_lhsT(lesT)(c)) for c in range(4)]}
    e_hnat = {(h, 0): lesp[:, h * 512:(h + 1) * 512] for h in range(2)}
    e_out = outp.tile([128, 1024], F32, tag="out", name="e_out")
    emit_gru("e", e_ih, e_hh, e_hnat, {0: e_out})
    for h in range(2):
        nc.sync.dma_start(oe[h * 128:(h + 1) * 128, :], e_out[:, h * 512:(h + 1) * 512])

    # ---- listener GRU ----
    l_ih = [(wlih, c * 128, (lambda c_: lambda h: uT_lhsT(c_, h))(c)) for c in range(4)]
    l_ih += [(wlih, 512 + c * 128, pair_lhsT(ss2T)(c)) for c in range(4)]
    l_hh = {p: [(wlhh, c * 128, pair_lhsT(lpsT[p])(c)) for c in range(4)]
            for p in range(P)}
    cl = {p: outp.tile([128, 1024], F32, tag="out", name=f"cl{p}") for p in range(P)}
    emit_gru("l", l_ih, l_hh, p_hnat, cl)

    # ---- scatter-blend cur_personal and store ----
    for p in range(P):
        for h in range(2):
            hsl = slice(h * 512, (h + 1) * 512)
            dd = gp.tile([128, 512], F32, tag="gate", name=f"bl_d{h}{p}")
            nc.vector.tensor_sub(dd, cs[p][:, hsl], cl[p][:, hsl])
            o = gp.tile([128, 512], F32, tag="gate", name=f"bl_o{h}{p}")
            nc.vector.scalar_tensor_tensor(
                out=o, in0=dd, scalar=pmcol(h, p), in1=cl[p][:, hsl],
                op0=AX.mult, op1=AX.add,
            )
            nc.sync.dma_start(op[h * 128:(h + 1) * 128, p, :], o)


def _build_program():
    nc = bass.Bass("TRN2", target_bir_lowering=False, debug=False)
    io = {}

    def din(name, shape):
        io[name] = nc.dram_tensor(name, list(shape), F32, kind="ExternalInput").ap()

    def dout(name, shape):
        io[name] = nc.dram_tensor(name, list(shape), F32, kind="ExternalOutput").ap()

    din("hist", (NCH, 2, 128, TC, G))
    din("uT", (U, BL))
    din("pm", (BL, P))
    din("lps", (BL, P, DP))
    din("les", (BL, E))
    din("wgih", (U + DP, H3))
    din("wghh", (G, H3))
    din("wpih", (U + G, H3))
    din("wphh", (DP, H3))
    din("wlih", (U + DP, H3))
    din("wlhh", (DP, H3))
    din("weih", (DP, H3))
    din("wehh", (E, H3))
    din("wbc", (128, G))
    dout("og", (BL, G))
    dout("op", (BL, P, DP))
    dout("oe", (BL, E))
    dout("oa", (BL, T))

    from contextlib import ExitStack
    with tile.TileContext(nc) as tc:
        with ExitStack() as ctx:
            _emit(nc, tc, ctx, io)
    if SPLIT_WAITS:
        _split_excess_waits(nc)
    return nc


_NC = None


def _get_program():
    global _NC
    if _NC is None:
        _NC = _build_program()
    return _NC


def kernel(utterance, party_mask, global_hist, last_personal_state,
           last_emotion_state,
           wg_ih, wg_hh, bg_ih, bg_hh, wp_ih, wp_hh, bp_ih, bp_hh,
           wl_ih, wl_hh, bl_ih, bl_hh, we_ih, we_hh, be_ih, be_hh, attn_w):
    asnp = lambda x: np.ascontiguousarray(np.asarray(x), dtype=np.float32)
    utterance = asnp(utterance)
    party_mask = asnp(party_mask)
    global_hist = asnp(global_hist)
    lps = asnp(last_personal_state)
    les = asnp(last_emotion_state)
    for b in (bg_ih, bg_hh, bp_ih, bp_hh, bl_ih, bl_hh, be_ih, be_hh):
        if np.abs(np.asarray(b)).max() != 0.0:
            raise NotImplementedError("nonzero GRU biases not supported")

    shared = {
        "wgih": asnp(np.asarray(wg_ih).T),
        "wghh": asnp(np.asarray(wg_hh).T),
        "wpih": asnp(np.asarray(wp_ih).T),
        "wphh": asnp(np.asarray(wp_hh).T),
        "wlih": asnp(np.asarray(wl_ih).T),
        "wlhh": asnp(np.asarray(wl_hh).T),
        "weih": asnp(np.asarray(we_ih).T),
        "wehh": asnp(np.asarray(we_hh).T),
        "wbc": asnp(np.tile(np.asarray(attn_w).reshape(1, G), (128, 1))),
    }

    in_maps = []
    for c in range(NCORES):
        sl = slice(c * BL, (c + 1) * BL)
        m = dict(shared)
        m["hist"] = asnp(
            global_hist[:, sl, :]
            .reshape(NCH, TC, 2, 128, G).transpose(0, 2, 3, 1, 4))
        m["uT"] = asnp(utterance[sl].T)
        m["pm"] = asnp(party_mask[sl])
        m["lps"] = asnp(lps[sl])
        m["les"] = asnp(les[sl])
        in_maps.append(m)

    nc = _get_program()
    from concourse.bass_utils import run_bass_kernel_spmd
    res = run_bass_kernel_spmd(
        nc, in_maps, core_ids=list(range(NCORES)), trace=TRACE
    )
    kernel.last_result = res

    og = np.concatenate([res.results[c]["og"] for c in range(NCORES)], axis=0)
    opd = np.concatenate([res.results[c]["op"] for c in range(NCORES)], axis=0)
    oe = np.concatenate([res.results[c]["oe"] for c in range(NCORES)], axis=0)
    oa = np.concatenate([res.results[c]["oa"] for c in range(NCORES)], axis=0)
    return og, opd, oe, oa[:, None, :]


# revision 22
# speedup vs baseline: 1.3353x; 1.0332x over previous
"""Trainium2 Bass kernel for nn_DialogueRNNCell (data-parallel over batch, 8 cores).

Sharding: batch axis (2048) split into 8 slices of 256 rows; GRU weights and the
attention vector are replicated.  Each core runs an identical Bass/Tile program.

Per-core layout: batch rows on SBUF partitions, two half-tiles of 128 rows.
The 64 MB/core global_hist stream is read once: a fused DVE scalar_tensor_tensor
computes the attention scores, and the softmax-weighted pooling accumulates in
PSUM via diag(exp(s_t)) @ hist_t matmuls (float32r, full PE rate).
"""

import sys

for _p in ("/opt/trn_rl_repo",):
    if _p not in sys.path:
        sys.path.insert(0, _p)

import numpy as np

import concourse.bass as bass
import concourse.tile as tile
from concourse import mybir
from concourse.vector_clock import ScopedClock

# ---- problem constants (hardcoded per spec) ----
B, P, T = 2048, 2, 128
U = G = DP = E = 512
H3 = 3 * 512
NCORES = 8
BL = B // NCORES  # 256 batch rows per core
TC = 2            # time steps per stream chunk
NCH = T // TC
F32 = mybir.dt.float32
F32R = mybir.dt.float32r
AX = mybir.AluOpType
AF = mybir.ActivationFunctionType

TRACE = False  # test harness can flip this to capture an NTFF profile

# free-dim offsets inside the packed `misc` tile [128, 1024]
O_ID = 0      # identity [128, 128]
O_WBC = 128   # attn_w broadcast [128, 512]
O_E = 640     # exp(scores) per half: [640:768] h=0, [768:896] h=1
O_PM = 896    # party mask cols: 896+h*2+p
O_DEN = 900   # 900+h
O_REC = 902   # 902+h
O_S = 904     # score scratch: 904 + h*4 + (c%2)*2 + i   (TC=2, chunk pairs)


# ---------------------------------------------------------------------------
# Workaround: walrus in this toolchain accepts only ONE sync wait on the
# CTRL-class drain emitted at TileContext exit.  Split the waits across a
# chain of drains.
# ---------------------------------------------------------------------------
def _patched_drain_and_barrier(self, tick_clock, wait_clock):
    nc = self.nc
    drain_inst = nc.sync.drain()
    wait_clock.add_sem_waits(
        drain_inst.ins, ScopedClock({None: tick_clock.global_clock})
    )
    inst = drain_inst.ins
    si = inst.sync_info
    maxw = 1
    if si is not None and len(si.on_wait) > maxw:
        waits = list(si.on_wait)
        SI = type(si)
        inst.sync_info = SI(on_wait=waits[:maxw], on_update=list(si.on_update))
        for k in range(maxw, len(waits), maxw):
            d2 = nc.sync.drain()
            d2.ins.sync_info = SI(on_wait=waits[k:k + maxw], on_update=[])
    nc.all_engine_barrier()
    assert self.sems is not None
    popped = nc._tile_sem_poison_stack.pop()
    assert popped is self._sem_poison
    nc.clear_and_free_semaphores(list(self.sems.allocated().values()))
    nc.all_engine_barrier()


tile.TileContext._drain_and_barrier = _patched_drain_and_barrier

_MAX_WAITS = 1
SPLIT_WAITS = True  # dev CoreSim runs need this off (NoOps lack fake sem updates)


def _split_excess_waits(nc, maxw=_MAX_WAITS):
    """This toolchain's walrus rejects instructions with more than `maxw`
    sync waits.  Move the excess onto NoOp instructions inserted directly
    before the offender on the same engine (waits are monotonic within a
    kernel, so waiting earlier in program order is safe)."""
    nseq = [0]
    for f in nc.m.functions:
        for blk in f.blocks:
            insts = list(blk.instructions)
            out = []
            changed = False
            for inst in insts:
                si = getattr(inst, "sync_info", None)
                if si is not None and len(si.on_wait) > maxw:
                    changed = True
                    waits = list(si.on_wait)
                    SI = type(si)
                    keep = len(waits) - maxw
                    for k in range(0, keep, maxw):
                        nop = mybir.InstNoOp(
                            name=f"waitnop_{nseq[0]}", ins=[], outs=[],
                            engine=inst.engine,
                        )
                        nseq[0] += 1
                        nop.sync_info = SI(on_wait=waits[k:k + maxw], on_update=[])
                        out.append(nop)
                    inst.sync_info = SI(on_wait=waits[keep:],
                                        on_update=list(si.on_update))
                out.append(inst)
            if changed:
                blk.instructions = out


# ---------------------------------------------------------------------------
# Device program
# ---------------------------------------------------------------------------
def _emit(nc, tc, ctx, io):
    from concourse.masks import make_identity

    hist, uT, pm, lps, les = io["hist"], io["uT"], io["pm"], io["lps"], io["les"]
    wgih, wghh, wpih, wphh, wlih, wlhh, weih, wehh, wbc = (
        io["wgih"], io["wghh"], io["wpih"], io["wphh"],
        io["wlih"], io["wlhh"], io["weih"], io["wehh"], io["wbc"],
    )
    og, op, oe, oa = io["og"], io["op"], io["oe"], io["oa"]

    pers = ctx.enter_context(tc.tile_pool(name="pers", bufs=1))
    histp = ctx.enter_context(tc.tile_pool(name="histp", bufs=8))
    scr = ctx.enter_context(tc.tile_pool(name="scr", bufs=1))
    dgp = ctx.enter_context(tc.tile_pool(name="dgp", bufs=2))
    wp = ctx.enter_context(tc.tile_pool(name="wp", bufs=14))
    xtp = ctx.enter_context(tc.tile_pool(name="xtp", bufs=5))
    gp = ctx.enter_context(tc.tile_pool(name="gp", bufs=9))
    outp = ctx.enter_context(tc.tile_pool(name="outp", bufs=6))
    nump = ctx.enter_context(tc.tile_pool(name="nump", bufs=2, space="PSUM"))
    gpsum = ctx.enter_context(tc.tile_pool(name="gpsum", bufs=4, space="PSUM"))
    trp = ctx.enter_context(tc.tile_pool(name="trp", bufs=1, space="PSUM"))

    # ---- packed constants / persistent state ----
    misc = pers.tile([128, 1024], F32, tag="misc", name="misc")
    ident = misc[:, O_ID:O_ID + 128]
    make_identity(nc, ident)
    nc.sync.dma_start(misc[:, O_WBC:O_WBC + G], wbc[:, :])
    for h in range(2):
        hs = h * 128
        for p in range(P):
            nc.sync.dma_start(
                misc[:, O_PM + h * 2 + p:O_PM + h * 2 + p + 1],
                pm[hs:hs + 128, p:p + 1],
            )

    etile = pers.tile([128, 256], F32, tag="etile", name="etile")
    sdr = pers.tile([128, 16], F32, tag="sdr", name="sdr")

    def Ecol(h, t):
        return etile[:, h * 128 + t:h * 128 + t + 1]

    def Eslice(h):
        return etile[:, h * 128:(h + 1) * 128]

    def pmcol(h, p):
        return misc[:, O_PM + h * 2 + p:O_PM + h * 2 + p + 1]

    hprev = pers.tile([128, 1024], F32, tag="hprev", name="hprev")
    lesp = pers.tile([128, 1024], F32, tag="lesp", name="lesp")
    lpsp = [pers.tile([128, 1024], F32, tag=f"lps{p}", name=f"lps{p}")
            for p in range(P)]
    for h in range(2):
        hs = h * 128
        nc.sync.dma_start(hprev[:, h * 512:(h + 1) * 512],
                          hist[NCH - 1, h, :, TC - 1, :])
        nc.sync.dma_start(lesp[:, h * 512:(h + 1) * 512], les[hs:hs + 128, :])
        for p in range(P):
            nc.sync.dma_start(lpsp[p][:, h * 512:(h + 1) * 512], lps[hs:hs + 128, p, :])

    uTt = pers.tile([128, 1024], F32, tag="uT", name="uTt")
    nc.sync.dma_start(
        uTt.rearrange("p (c b) -> p c b", c=4).bitcast(F32R),
        uT.rearrange("(c p) b -> p c b", p=128).bitcast(F32R),
    )

    def uT_lhsT(c, h):
        return uTt[:, c * 256 + h * 128:c * 256 + h * 128 + 128]

    num = [nump.tile([128, G], F32, tag="num", name=f"num{h}") for h in range(2)]
    num_started = [False, False]

    # ---- helpers ----
    def transpose_pair(srcs, name):
        """srcs[h] = [128b, 512d] SBUF -> pair tile [128, 1024]:
        [:, h*512 + c*128 : ...] = chunk-c lhsT for half h."""
        dst = xtp.tile([128, 1024], F32, tag="xT", name=f"xT_{name}")
        for h in range(2):
            pt = trp.tile([128, 512], F32, tag="tr", name=f"tr_{name}{h}")
            for c in range(4):
                nc.tensor.transpose(
                    pt[:, c * 128:(c + 1) * 128],
                    srcs[h][:, c * 128:(c + 1) * 128], ident,
                )
            # rounds to f32r so the consuming matmuls pass BIR verification
            nc.scalar.copy(dst[:, h * 512:(h + 1) * 512].bitcast(F32R), pt)
        return dst

    def pair_lhsT(pair):
        return lambda c: lambda h: pair[:, h * 512 + c * 128:h * 512 + (c + 1) * 128]

    _wcache = {}

    def wtile(wdram, r0, n0):
        key = (id(wdram), r0, n0)
        if key in _wcache:
            return _wcache[key]
        t = wp.tile([128, 512], F32, tag="w", name=f"w_{len(_wcache)}")
        nc.sync.dma_start(t.bitcast(F32R),
                          wdram[r0:r0 + 128, n0:n0 + 512].bitcast(F32R))
        _wcache[key] = t
        return t

    def stream_pair(k):
        """Two chunks (4 time steps) per emission: scores on DVE, one batched
        exp on ACT, diag built on ACT, weighted accumulate on PE."""
        hts = {}
        for j in range(2):
            c = 2 * k + j
            for h in range(2):
                ht = histp.tile([128, TC, G], F32, tag="hist", name=f"ht{c}_{h}")
                nc.sync.dma_start(ht.bitcast(F32R), hist[c, h].bitcast(F32R))
                hts[j, h] = ht
                junk = scr.tile([128, 1024], F32, tag="junk", name=f"junk{c}_{h}")
                s0 = 8 * (k % 2) + h * 4 + j * 2
                for i in range(TC):
                    nc.vector.scalar_tensor_tensor(
                        out=junk[:, i * 512:(i + 1) * 512], in0=ht[:, i, :],
                        scalar=1.0, in1=misc[:, O_WBC:O_WBC + G],
                        op0=AX.mult, op1=AX.mult,
                        accum_out=sdr[:, s0 + i:s0 + i + 1],
                    )
        for h in range(2):
            nc.scalar.activation(
                etile[:, h * 128 + 4 * k:h * 128 + 4 * k + 4],
                sdr[:, 8 * (k % 2) + h * 4:8 * (k % 2) + h * 4 + 4], AF.Exp,
            )
        for h in range(2):
            dg = dgp.tile([128, 4, 128], F32, tag="diag", name=f"dg{k}_{h}")
            for j in range(2):
                for i in range(TC):
                    t = 4 * k + j * 2 + i
                    nc.scalar.activation(dg[:, j * 2 + i, :].bitcast(F32R), ident,
                                         AF.Copy, scale=Ecol(h, t))
                    nc.tensor.matmul(
                        num[h], lhsT=dg[:, j * 2 + i, :].bitcast(F32R),
                        rhs=hts[j, h][:, i, :].bitcast(F32R),
                        start=(not num_started[h]),
                        stop=(k == T // 4 - 1 and j == 1 and i == TC - 1),
                        skip_group_check=True,
                    )
                    num_started[h] = True

    def gru_phases(prefix, ih_srcs, hh_srcs_by_party, h_nat, out_pair_of_p,
                   ih_late=None):
        """One GRU cell as a list of phase closures so the caller can
        interleave them with stream pairs (keeps the PE queue short).
        `ih_late`: ih sources only available at the tail (e.g. pooled); when
        given, the r phase is split into an early part (pre-issued during the
        stream) and a late finisher."""
        parties = sorted(hh_srcs_by_party)
        ih_late = ih_late or []
        ih_all = ih_srcs + ih_late
        st = {"r": {}, "z": {}, "c": None}
        total = {p: len(ih_all) + len(hh_srcs_by_party[p]) for p in parties}

        def rz_alloc(gname):
            ps, nmm = {}, {}
            for h in range(2):
                for p in parties:
                    ps[h, p] = gpsum.tile([128, 512], F32, tag="gps",
                                          name=f"{prefix}_{gname}{h}{p}")
                    nmm[h, p] = 0
            st[gname + "_ps"] = (ps, nmm)
            return ps, nmm

        def rz_mms(gname, n0, ih_part, with_hh):
            ps, nmm = st[gname + "_ps"]

            def mm(h, p, lh, wt):
                nc.tensor.matmul(
                    ps[h, p], lhsT=lh.bitcast(F32R), rhs=wt.bitcast(F32R),
                    start=(nmm[h, p] == 0), stop=(nmm[h, p] == total[p] - 1),
                    skip_group_check=True,
                )
                nmm[h, p] += 1

            for wdram, r0, lfn in ih_part:
                wt = wtile(wdram, r0, n0)
                for h in range(2):
                    for p in parties:
                        mm(h, p, lfn(h), wt)
            if with_hh:
                for p in parties:
                    for wdram, r0, lfn in hh_srcs_by_party[p]:
                        wt = wtile(wdram, r0, n0)
                        for h in range(2):
                            mm(h, p, lfn(h), wt)

        def rz_sigmoids(gname):
            ps, _ = st[gname + "_ps"]
            for p in parties:
                g = gp.tile([128, 1024], F32, tag="gate",
                            name=f"{prefix}_{gname}sb{p}")
                for h in range(2):
                    nc.scalar.activation(g[:, h * 512:(h + 1) * 512], ps[h, p],
                                         AF.Sigmoid)
                st[gname][p] = g

        def rz_phase(gname, n0):
            rz_alloc(gname)
            rz_mms(gname, n0, ih_all, True)
            rz_sigmoids(gname)

        def ni_phase():
            cps = {h: gpsum.tile([128, 512], F32, tag="gps",
                                 name=f"{prefix}_c{h}") for h in range(2)}
            for idx, (wdram, r0, lfn) in enumerate(ih_all):
                wt = wtile(wdram, r0, 1024)
                for h in range(2):
                    nc.tensor.matmul(
                        cps[h], lhsT=lfn(h).bitcast(F32R), rhs=wt.bitcast(F32R),
                        start=(idx == 0), stop=(idx == len(ih_all) - 1),
                        skip_group_check=True,
                    )
            st["c"] = cps

        def nh_phase():
            cps = st["c"]
            for p in parties:
                dps = {h: gpsum.tile([128, 512], F32, tag="gps",
                                     name=f"{prefix}_d{h}{p}") for h in range(2)}
                srcs = hh_srcs_by_party[p]
                for idx, (wdram, r0, lfn) in enumerate(srcs):
                    wt = wtile(wdram, r0, 1024)
                    for h in range(2):
                        nc.tensor.matmul(
                            dps[h], lhsT=lfn(h).bitcast(F32R), rhs=wt.bitcast(F32R),
                            start=(idx == 0), stop=(idx == len(srcs) - 1),
                            skip_group_check=True,
                        )
                for h in range(2):
                    hsl = slice(h * 512, (h + 1) * 512)
                    t1 = gp.tile([128, 512], F32, tag="gate",
                                 name=f"{prefix}_t1{h}{p}")
                    nc.vector.tensor_mul(t1, st["r"][p][:, hsl], dps[h])
                    nin = gp.tile([128, 512], F32, tag="gate",
                                  name=f"{prefix}_ni{h}{p}")
                    nc.vector.tensor_add(nin, t1, cps[h])
                    n_t = gp.tile([128, 512], F32, tag="gate",
                                  name=f"{prefix}_n{h}{p}")
                    nc.scalar.activation(n_t, nin, AF.Tanh)
                    d2 = gp.tile([128, 512], F32, tag="gate",
                                 name=f"{prefix}_d2{h}{p}")
                    nc.vector.tensor_sub(d2, h_nat[h, p], n_t)
                    m = gp.tile([128, 512], F32, tag="gate",
                                name=f"{prefix}_m{h}{p}")
                    nc.vector.tensor_mul(m, st["z"][p][:, hsl], d2)
                    nc.vector.tensor_add(out_pair_of_p[p][:, hsl], n_t, m)

        if ih_late:
            def r_early():
                rz_alloc("r")
                rz_mms("r", 0, ih_srcs, True)

            def r_late():
                rz_mms("r", 0, ih_late, False)
                rz_sigmoids("r")

            return [r_early, r_late, lambda: rz_phase("z", 512),
                    ni_phase, nh_phase]
        return [lambda: rz_phase("r", 0), lambda: rz_phase("z", 512),
                ni_phase, nh_phase]

    def emit_gru(prefix, ih_srcs, hh_srcs_by_party, h_nat, out_pair_of_p):
        for ph in gru_phases(prefix, ih_srcs, hh_srcs_by_party, h_nat,
                             out_pair_of_p):
            ph()

    def onehot_blend(a_of_hp, name):
        """pair tile: [:, h*512:...] = sum_p pm[h,p] * a_of_hp(h,p)"""
        res = outp.tile([128, 1024], F32, tag="out", name=f"{name}_pair")
        for h in range(2):
            hsl = slice(h * 512, (h + 1) * 512)
            tmp = gp.tile([128, 512], F32, tag="gate", name=f"{name}_t{h}")
            nc.vector.tensor_scalar_mul(tmp, a_of_hp(h, 1), pmcol(h, 1))
            nc.vector.scalar_tensor_tensor(
                out=res[:, hsl], in0=a_of_hp(h, 0), scalar=pmcol(h, 0), in1=tmp,
                op0=AX.mult, op1=AX.add,
            )
        return res

    # ---- emission: stream pairs with the global GRU + transposes woven in ---
    stream_pair(0)
    ss = onehot_blend(lambda h, p: lpsp[p][:, h * 512:(h + 1) * 512], "ss")
    ssT = transpose_pair([ss[:, 0:512], ss[:, 512:1024]], "ssT")
    hprevT = transpose_pair([hprev[:, 0:512], hprev[:, 512:1024]], "hpT")
    stream_pair(1)
    g_ih = [(wgih, c * 128, (lambda c_: lambda h: uT_lhsT(c_, h))(c)) for c in range(4)]
    g_ih += [(wgih, 512 + c * 128, pair_lhsT(ssT)(c)) for c in range(4)]
    g_hh = {0: [(wghh, c * 128, pair_lhsT(hprevT)(c)) for c in range(4)]}
    g_hnat = {(h, 0): hprev[:, h * 512:(h + 1) * 512] for h in range(2)}
    g_out = outp.tile([128, 1024], F32, tag="out", name="g_out")
    g_phases = gru_phases("g", g_ih, g_hh, g_hnat, {0: g_out})
    lpsT = [None, None]
    lesT = None
    p_phases = [None]
    for k in range(2, T // 4):
        stream_pair(k)
        if k - 2 < len(g_phases):
            g_phases[k - 2]()
            if k - 2 == len(g_phases) - 1:
                for h in range(2):
                    nc.sync.dma_start(og[h * 128:(h + 1) * 128, :],
                                      g_out[:, h * 512:(h + 1) * 512])
        elif k == 7:
            lpsT[0] = transpose_pair([lpsp[0][:, 0:512], lpsp[0][:, 512:1024]],
                                     "lpsT0")
            lpsT[1] = transpose_pair([lpsp[1][:, 0:512], lpsp[1][:, 512:1024]],
                                     "lpsT1")
            lesT = transpose_pair([lesp[:, 0:512], lesp[:, 512:1024]], "lesT")
        elif k == 9:
            p_ih = [(wpih, c * 128, (lambda c_: lambda h: uT_lhsT(c_, h))(c))
                    for c in range(4)]
            p_late = None  # bound after pooledT exists
            p_hh = {p: [(wphh, c * 128, pair_lhsT(lpsT[p])(c)) for c in range(4)]
                    for p in range(P)}
            p_hnat = {(h, p): lpsp[p][:, h * 512:(h + 1) * 512]
                      for h in range(2) for p in range(P)}
            cs = {p: outp.tile([128, 1024], F32, tag="out", name=f"cs{p}")
                  for p in range(P)}
            pooledT_box = [None]
            p_late = [(wpih, 512 + c * 128,
                       (lambda c_: lambda h: pooledT_box[0][
                           :, h * 512 + c_ * 128:h * 512 + (c_ + 1) * 128])(c))
                      for c in range(4)]
            p_phases = gru_phases("p", p_ih, p_hh, p_hnat, cs, ih_late=p_late)
        elif k == T // 4 - 4:
            p_phases[0]()  # personal r: pooled-independent matmuls

    # ---- attention epilogue: pooled + alpha ----
    drr = pers.tile([128, 4], F32, tag="drr", name="drr")
    alpha = gp.tile([128, 256], F32, tag="gate", name="alpha")
    pooled = gp.tile([128, 1024], F32, tag="gate", name="pooled")
    for h in range(2):
        den = drr[:, h:h + 1]
        nc.vector.tensor_reduce(out=den, in_=Eslice(h), axis=mybir.AxisListType.X,
                                op=AX.add)
        rec = drr[:, 2 + h:3 + h]
        nc.vector.reciprocal(rec, den)
        nc.vector.tensor_scalar_mul(alpha[:, h * 128:(h + 1) * 128], Eslice(h), rec)
        nc.sync.dma_start(oa[h * 128:(h + 1) * 128, :], alpha[:, h * 128:(h + 1) * 128])
        nc.vector.tensor_scalar_mul(pooled[:, h * 512:(h + 1) * 512], num[h], rec)
    pooledT = transpose_pair([pooled[:, 0:512], pooled[:, 512:1024]], "plT")
    pooledT_box[0] = pooledT

    # ---- personal (speaker) GRU (r phase was pre-issued in-stream) ----
    for ph in p_phases[1:]:
        ph()

    # ss2 = speaker row of cur_speaker (= emotion input; speaker rows of
    # cur_personal)
    ss2 = onehot_blend(lambda h, p: cs[p][:, h * 512:(h + 1) * 512], "ss2")
    ss2T = transpose_pair([ss2[:, 0:512], ss2[:, 512:1024]], "ss2T")

    # ---- emotion GRU (depends only on ss2) ----
    e_ih = [(weih, c * 128, pair_lhsT(ss2T)(c)) for c in range(4)]
    e_hh = {0: [(wehh, c * 128, pair_lhsT(lesT)(c)) for c in range(4)]}
    e_hnat = {(h, 0): lesp[:, h * 512:(h + 1) * 512] for h in range(2)}
    e_out = outp.tile([128, 1024], F32, tag="out", name="e_out")
    emit_gru("e", e_ih, e_hh, e_hnat, {0: e_out})
    for h in range(2):
        nc.sync.dma_start(oe[h * 128:(h + 1) * 128, :], e_out[:, h * 512:(h + 1) * 512])

    # ---- listener GRU ----
    l_ih = [(wlih, c * 128, (lambda c_: lambda h: uT_lhsT(c_, h))(c)) for c in range(4)]
    l_ih += [(wlih, 512 + c * 128, pair_lhsT(ss2T)(c)) for c in range(4)]
    l_hh = {p: [(wlhh, c * 128, pair_lhsT(lpsT[p])(c)) for c in range(4)]
            for p in range(P)}
    cl = {p: outp.tile([128, 1024], F32, tag="out", name=f"cl{p}") for p in range(P)}
    emit_gru("l", l_ih, l_hh, p_hnat, cl)

    # ---- scatter-blend cur_personal and store ----
    for p in range(P):
        for h in range(2):
            hsl = slice(h * 512, (h + 1) * 512)
            dd = gp.tile([128, 512], F32, tag="gate", name=f"bl_d{h}{p}")
            nc.vector.tensor_sub(dd, cs[p][:, hsl], cl[p][:, hsl])
            o = gp.tile([128, 512], F32, tag="gate", name=f"bl_o{h}{p}")
            nc.vector.scalar_tensor_tensor(
                out=o, in0=dd, scalar=pmcol(h, p), in1=cl[p][:, hsl],
                op0=AX.mult, op1=AX.add,
            )
            nc.sync.dma_start(op[h * 128:(h + 1) * 128, p, :], o)


def _build_program():
    nc = bass.Bass("TRN2", target_bir_lowering=False, debug=False)
    io = {}

    def din(name, shape):
        io[name] = nc.dram_tensor(name, list(shape), F32, kind="ExternalInput").ap()

    def dout(name, shape):
        io[name] = nc.dram_tensor(name, list(shape), F32, kind="ExternalOutput").ap()

    din("hist", (NCH, 2, 128, TC, G))
    din("uT", (U, BL))
    din("pm", (BL, P))
    din("lps", (BL, P, DP))
    din("les", (BL, E))
    din("wgih", (U + DP, H3))
    din("wghh", (G, H3))
    din("wpih", (U + G, H3))
    din("wphh", (DP, H3))
    din("wlih", (U + DP, H3))
    din("wlhh", (DP, H3))
    din("weih", (DP, H3))
    din("wehh", (E, H3))
    din("wbc", (128, G))
    dout("og", (BL, G))
    dout("op", (BL, P, DP))
    dout("oe", (BL, E))
    dout("oa", (BL, T))

    from contextlib import ExitStack
    with tile.TileContext(nc) as tc:
        with ExitStack() as ctx:
            _emit(nc, tc, ctx, io)
    if SPLIT_WAITS:
        _split_excess_waits(nc)
    return nc


_NC = None


def _get_program():
    global _NC
    if _NC is None:
        _NC = _build_program()
    return _NC


def kernel(utterance, party_mask, global_hist, last_personal_state,
           last_emotion_state,
           wg_ih, wg_hh, bg_ih, bg_hh, wp_ih, wp_hh, bp_ih, bp_hh,
           wl_ih, wl_hh, bl_ih, bl_hh, we_ih, we_hh, be_ih, be_hh, attn_w):
    asnp = lambda x: np.ascontiguousarray(np.asarray(x), dtype=np.float32)
    utterance = asnp(utterance)
    party_mask = asnp(party_mask)
    global_hist = asnp(global_hist)
    lps = asnp(last_personal_state)
    les = asnp(last_emotion_state)
    for b in (bg_ih, bg_hh, bp_ih, bp_hh, bl_ih, bl_hh, be_ih, be_hh):
        if np.abs(np.asarray(b)).max() != 0.0:
            raise NotImplementedError("nonzero GRU biases not supported")

    shared = {
        "wgih": asnp(np.asarray(wg_ih).T),
        "wghh": asnp(np.asarray(wg_hh).T),
        "wpih": asnp(np.asarray(wp_ih).T),
        "wphh": asnp(np.asarray(wp_hh).T),
        "wlih": asnp(np.asarray(wl_ih).T),
        "wlhh": asnp(np.asarray(wl_hh).T),
        "weih": asnp(np.asarray(we_ih).T),
        "wehh": asnp(np.asarray(we_hh).T),
        "wbc": asnp(np.tile(np.asarray(attn_w).reshape(1, G), (128, 1))),
    }

    in_maps = []
    for c in range(NCORES):
        sl = slice(c * BL, (c + 1) * BL)
        m = dict(shared)
        m["hist"] = asnp(
            global_hist[:, sl, :]
            .reshape(NCH, TC, 2, 128, G).transpose(0, 2, 3, 1, 4))
        m["uT"] = asnp(utterance[sl].T)
        m["pm"] = asnp(party_mask[sl])
        m["lps"] = asnp(lps[sl])
        m["les"] = asnp(les[sl])
        in_maps.append(m)

    nc = _get_program()
    from concourse.bass_utils import run_bass_kernel_spmd
    res = run_bass_kernel_spmd(
        nc, in_maps, core_ids=list(range(NCORES)), trace=TRACE
    )
    kernel.last_result = res

    og = np.concatenate([res.results[c]["og"] for c in range(NCORES)], axis=0)
    opd = np.concatenate([res.results[c]["op"] for c in range(NCORES)], axis=0)
    oe = np.concatenate([res.results[c]["oe"] for c in range(NCORES)], axis=0)
    oa = np.concatenate([res.results[c]["oa"] for c in range(NCORES)], axis=0)
    return og, opd, oe, oa[:, None, :]
